# revision 2
# baseline (speedup 1.0000x reference)
"""Fused cross-attention kernel for Trainium2 (Bass/Tile), 8-core SPMD.

Problem: query/key_value [T=4, B=2, C=128, H=32, W=32] -> tokens [B, N=4096, C],
QKV projections (128x128), full softmax attention over N tokens per batch.

Sharding: core = b*4 + t handles batch b, query tokens [t*1024, (t+1)*1024)
against all 4096 K/V tokens of batch b.

Algebraic restructure (vs. materializing Q/K/V):
  scores:  S^T[m,n] = x_kv[m] . qk[n]   with  qk = (Wk^T Wq) x_q + Wk^T bq,
           prescaled by PRE = scale*log2e*beta/8 (see below)
  output:  O^T = Wv Z / rowsum,  Z[c,n] = sum_m x_kv[m,c] P[m,n]

v2 speedups over the 28us baseline (which was simultaneously at the bf16
PE roofline and the ACT 1-elem/cycle exp roofline):
  1. Z matmul in fp8 e4m3 DoubleRow, m-chunks paired for 256-deep
     contraction: 2 output cols/cycle -> 8192 PE cycles instead of 32768.
     End-to-end rel err with p+kvxT in e4m3 measured 1.15e-2 (gate 2e-2);
     the p-quantization error largely cancels between numerator Z and
     denominator r because r is computed from the same quantized p8.
  2. exp split across TWO engines: ACT (spline exp, 1/cycle @1.2GHz) and a
     custom DVE op EXP8C_ANT: monic cubic in v then 3 squarings
     (8 ALU stages, 1 elem/cycle @0.96GHz).  p = h(v)^8 ~ 2^(8u-5) where
     v = beta*u, u = s*scale*log2e/8; the monic-normalization constant
     beta = (a3*2^(-5/8))^(1/3) is folded into the host-side qk prescale,
     and the 2^-5 fp8 range shift cancels in Z/r.  Poly minimax err 1.9e-3
     (x8 after squarings) is value-deterministic so it also cancels in
     Z/r: measured +0.04% end-to-end.
  3. rowsums as 256 tiny PE matmuls (p8-block stationary, FWL-loaded
     behind the S matmul streams; moving ones8 = 1 col): lands r in PSUM
     [128, 8nb, 32j] as independent single-shot groups (PSUM zero-region
     marking breaks interleaved accumulation groups on HW), reduced to
     [128, 8] by one DVE tensor_reduce at the end.
This rebalances to PE ~18us / ACT ~16us / DVE ~16us.

ldweights-only filler instructions keep the PE busy streak alive for the
HAM clock ramp (p-states 0.65/1.2/2.4 GHz).
"""

import math
from contextlib import ExitStack

import numpy as np
import ml_dtypes

import concourse.bass as bass
import concourse.mybir as mybir
import concourse.tile as tile
from concourse import bacc
from concourse.bass_utils import run_bass_kernel_spmd

F32 = mybir.dt.float32
BF16 = mybir.dt.bfloat16
E4 = mybir.dt.float8e4
AF = mybir.ActivationFunctionType
e4np = ml_dtypes.float8_e4m3
bfnp = ml_dtypes.bfloat16

C = 128        # model dim
NQ = 1024      # query tokens per core
M = 4096       # kv tokens per batch
NCH = M // 128 # m chunks
NPAIR = NCH // 2
T = 4
B = 2
SCALE = 1.0 / math.sqrt(float(C))
N_CORES = 8
NRS = NQ // C  # rowsum cols appended to out2

CFG = dict(
    dve_pairs=15,   # pairs whose odd chunk exps on DVE (rest on ACT)
    p_bufs=4,       # p8 pair-tile SBUF buffers
    ps_s_bufs=2,    # score PSUM buffers ([128,1024] = 2 banks each)
    pe_warm=24,     # ldweights warm-ups during the DMA window
    fillers=1,      # ldweights fillers per pair (hold the PE HAM streak)
    head_fill=8,    # ldweights fillers between qk proj and chunk 0
    unroll=8,       # kernel bodies per For_i iteration (amortizes barrier)
)

_NC = None

# ---------------- custom DVE exp op ----------------
from concourse.dve_spec import Spec, Src0, C0, C1, C2, lower as _dve_lower
from concourse.dve_uop import DveOpSpec
from concourse.dve_ops import DveOp, OPS as _DVE_OPS, CUSTOM_DVE_SPECS

_FIT_R = 1.13


def _fit_cubic():
    us = np.linspace(-_FIT_R, _FIT_R, 4001)
    tgt = np.exp2(us)
    w = np.ones_like(us)
    for _ in range(60):
        c = np.polyfit(us, tgt, 3, w=w)
        r = np.abs(np.polyval(c, us) / tgt - 1)
        w *= (1 + r / r.max())
        w /= w.mean()
    return [float(x) for x in c]


_A3, _A2, _A1, _A0 = _fit_cubic()
_KAPPA = 2.0 ** (-5.0 / 8.0)
BETA = (_A3 * _KAPPA) ** (1.0 / 3.0)
_EC2 = _KAPPA * _A2 / BETA**2
_EC1 = _KAPPA * _A1 / BETA
_EC0 = _KAPPA * _A0
ACT_SCALE = 8.0 * math.log(2.0) / BETA
ACT_BIAS = -5.0 * math.log(2.0)
PRE = SCALE * math.log2(math.e) * BETA / 8.0  # host fold into qk


def _exp_ref(in0, in1, s0, s1, imm2):
    v = np.asarray(in0, np.float32)
    h = ((v + imm2) * v + s1) * v + s0
    h2 = h * h
    h4 = h2 * h2
    return (h4 * h4).astype(np.float32)


def _make_exp_op():
    import concourse.dve_ops as _do
    name = "EXP8C_ANT"
    for op in _DVE_OPS:
        if op.name == name:
            return op
    v = Src0
    h = ((v + C2) * v + C1) * v + C0
    h2 = h * h
    h4 = h2 * h2
    spec = Spec(body=h4 * h4, reference=_exp_ref)
    shas = {}
    for ver in ("v3", "v4"):
        uops = _dve_lower(spec, ver=ver)
        shas[ver] = DveOpSpec(name=name, opcode=1, uops=uops,
                              rd1_en=False).sha(ver)
    op = DveOp(name, spec, subdim=False, uops_sha=shas)
    _DVE_OPS.append(op)
    CUSTOM_DVE_SPECS[name] = spec
    _do._SUB_OPCODE_FOR_NAME[name] = _do._CUSTOM_DVE_ROW_BASE + len(_DVE_OPS) - 1
    return op


EXP_OP = _make_exp_op()


def build_nc(reps=1, loop_reps=0, **overrides):
    cfg = dict(CFG)
    cfg.update(overrides)

    nc = bacc.Bacc()
    qx = nc.dram_tensor("qx", [C, NQ], BF16, kind="ExternalInput")
    aT = nc.dram_tensor("aT", [C, C], BF16, kind="ExternalInput")
    bqk = nc.dram_tensor("bqk", [C, 1], F32, kind="ExternalInput")
    wvT = nc.dram_tensor("wvT", [C, C], BF16, kind="ExternalInput")
    kvx = nc.dram_tensor("kvx", [C, M], BF16, kind="ExternalInput")
    kvxT8 = nc.dram_tensor("kvxT8", [128, NCH, 128], E4, kind="ExternalInput")
    out2 = nc.dram_tensor("out2", [C, NQ + NRS], F32, kind="ExternalOutput")

    unroll = cfg["unroll"]
    if loop_reps and loop_reps % unroll == 0 and loop_reps >= unroll:
        loop_iters, reps = loop_reps // unroll, reps * unroll
    elif loop_reps:
        loop_iters = loop_reps
    else:
        loop_iters = 0

    dve_on = {2 * k + 1 for k in range(cfg["dve_pairs"])}

    with tile.TileContext(nc) as tc, ExitStack() as ctx:
        const = ctx.enter_context(tc.tile_pool(name="const", bufs=1))
        proj = ctx.enter_context(tc.tile_pool(name="proj", bufs=1))
        pwork = ctx.enter_context(tc.tile_pool(name="pwork", bufs=cfg["p_bufs"]))
        owork = ctx.enter_context(tc.tile_pool(name="owork", bufs=2))
        psum = ctx.enter_context(tc.tile_pool(name="psum", bufs=2, space="PSUM"))

        def misc_tile(name):
            # borrow a rotating score-PSUM buffer for small/late matmuls
            return psum.tile([128, NQ], F32, tag="ps_s",
                             bufs=cfg["ps_s_bufs"], name=name)

        # Constants (gpsimd, no DMA deps). Warm the exp table first so the
        # table load overlaps the input DMAs / NEFF preamble.
        ones_f32 = const.tile([128, 1], F32)
        nc.gpsimd.memset(ones_f32, 1.0)
        warm = const.tile([128, 1], F32)
        nc.scalar.activation(warm, ones_f32, AF.Exp)
        ones8 = const.tile([128, 1], E4)
        nc.gpsimd.memset(ones8, 1.0)
        bias_t = const.tile([128, 1], F32)
        nc.gpsimd.memset(bias_t, ACT_BIAS)
        warm_w = const.tile([128, 128], BF16)
        nc.gpsimd.memset(warm_w, 1.0)

        # PE warm-up: ldweights-only ops to hold the HAM clock ramp.
        for _w in range(cfg["pe_warm"]):
            nc.tensor.ldweights(warm_w)

        # Input DMAs, spread across the SP and ACT HWDGE rings.
        qx_sb = const.tile([C, NQ], BF16)
        nc.sync.dma_start(qx_sb, qx[:])
        aT_sb = const.tile([C, C], BF16)
        nc.sync.dma_start(aT_sb, aT[:])
        bqk_sb = const.tile([C, 1], F32)
        nc.sync.dma_start(bqk_sb, bqk[:])
        wvT_sb = const.tile([C, C], BF16)
        nc.sync.dma_start(wvT_sb, wvT[:])
        kvx_sb = const.tile([C, M], BF16)
        nc.sync.dma_start(kvx_sb[:, 0:1024], kvx[:, 0:1024])
        nc.scalar.dma_start(kvx_sb[:, 1024:2560], kvx[:, 1024:2560])
        nc.scalar.dma_start(kvx_sb[:, 2560:4096], kvx[:, 2560:4096])
        kvxT8_sb = const.tile([128, NCH, 128], E4)
        nc.sync.dma_start(kvxT8_sb[:, 0:16, :], kvxT8[:, 0:16, :])
        nc.scalar.dma_start(kvxT8_sb[:, 16:32, :], kvxT8[:, 16:32, :])

        loop_cm = tc.For_i(0, loop_iters, 1) if loop_iters else None
        if loop_cm is not None:
            loop_cm.__enter__()
        for _rep in range(reps):
            # ---- qk projection: qk = (A*PRE) @ qx + bqk*PRE ----
            psq = misc_tile("psq")
            for h in range(2):
                nc.tensor.matmul(psq[:, h * 512:(h + 1) * 512], lhsT=aT_sb,
                                 rhs=qx_sb[:, h * 512:(h + 1) * 512],
                                 start=True, stop=True)
            qk_sb = proj.tile([C, NQ], BF16, name="qk_sb")
            with nc.allow_low_precision(reason="bf16 qk tokens"):
                nc.vector.tensor_scalar_add(qk_sb[:, 0:512], psq[:, 0:512],
                                            bqk_sb)
                nc.scalar.activation(qk_sb[:, 512:1024], psq[:, 512:1024],
                                     AF.Identity, bias=bqk_sb)
            for _f in range(cfg["head_fill"]):
                nc.tensor.ldweights(warm_w)

            # ---- pair loop: 2 m-chunks per iteration ----
            psz = psum.tile([128, NQ], F32, tag="ps_z", bufs=1, name="psz")
            psr = psum.tile([128, NRS, NCH], F32, tag="ps_r", bufs=1,
                            name="psr")
            for k in range(NPAIR):
                p8 = pwork.tile([128, 2, NQ], E4, tag="p8",
                                bufs=cfg["p_bufs"])
                for jj in range(2):
                    j = 2 * k + jj
                    pss = psum.tile([128, NQ], F32, tag="ps_s",
                                    bufs=cfg["ps_s_bufs"])
                    for h in range(2):
                        nc.tensor.matmul(
                            pss[:, h * 512:(h + 1) * 512],
                            lhsT=kvx_sb[:, j * 128:(j + 1) * 128],
                            rhs=qk_sb[:, h * 512:(h + 1) * 512],
                            start=True, stop=True)
                    with nc.allow_low_precision(reason="fp8 p"):
                        if j in dve_on:
                            nc.vector._custom_dve(
                                EXP_OP, out=p8[:, jj, :], in0=pss,
                                s0=_EC0, s1=_EC1, imm2=_EC2)
                        else:
                            nc.scalar.activation(p8[:, jj, :], pss, AF.Exp,
                                                 scale=ACT_SCALE, bias=bias_t)
                # Z: fp8 DoubleRow, 256-deep contraction per pair
                for h in range(2):
                    nc.tensor.matmul(
                        psz[:, h * 512:(h + 1) * 512],
                        lhsT=kvxT8_sb[:, 2 * k:2 * k + 2, :],
                        rhs=p8[:, :, h * 512:(h + 1) * 512],
                        start=(k == 0), stop=(k == NPAIR - 1),
                        perf_mode=mybir.MatmulPerfMode.DoubleRow)
                if cfg["fillers"]:
                    for _f in range(cfg["fillers"]):
                        # reading p8 pins the filler to this pair
                        nc.tensor.ldweights(p8[:, 0, 0:128])
                # rowsums: independent single-shot groups (see docstring)
                for jj in range(2):
                    j = 2 * k + jj
                    for nb in range(NRS):
                        nc.tensor.matmul(
                            psr[:, nb, j:j + 1],
                            lhsT=p8[:, jj, nb * 128:(nb + 1) * 128],
                            rhs=ones8,
                            start=True, stop=True, skip_group_check=True)

            # ---- O^T = Wv @ Z (unnormalized; host divides by rowsums),
            # quarter-split so drain/matmul/copy/DMA pipeline across engines.
            # Rowsums reduced into cols [NQ, NQ+8) of the same output. ----
            z_sb = proj.tile([C, NQ], BF16, name="z_sb")
            pso = misc_tile("pso")
            o_sb = owork.tile([128, NQ + NRS], F32, tag="o_sb", bufs=2,
                              name="o_sb")
            nc.vector.tensor_reduce(o_sb[:, NQ:NQ + NRS], psr[:, :, :],
                                    mybir.AxisListType.X, mybir.AluOpType.add)
            for q in range(4):
                qs = slice(q * 256, (q + 1) * 256)
                with nc.allow_low_precision(reason="bf16 z"):
                    nc.scalar.copy(z_sb[:, qs], psz[:, qs])
                nc.tensor.matmul(pso[:, qs], lhsT=wvT_sb, rhs=z_sb[:, qs],
                                 start=True, stop=True)
                nc.vector.tensor_copy(o_sb[:, qs], pso[:, qs])
                if q < 3:
                    nc.sync.dma_start(out2[:, qs], o_sb[:, qs])
                else:
                    nc.sync.dma_start(out2[:, 768:NQ + NRS],
                                      o_sb[:, 768:NQ + NRS])
        if loop_cm is not None:
            loop_cm.__exit__(None, None, None)
    nc.compile()
    return nc


def _prepare_in_maps(query, key_value, Wq, bq, Wk, bk, Wv, bv):
    q = np.asarray(query, np.float32)
    kv = np.asarray(key_value, np.float32)
    Wq64 = np.asarray(Wq, np.float64)
    Wk64 = np.asarray(Wk, np.float64)
    aT = np.ascontiguousarray(((Wq64.T @ Wk64) * PRE).astype(bfnp))
    bqk = np.ascontiguousarray(
        ((Wk64.T @ np.asarray(bq, np.float64)) * PRE
         ).astype(np.float32).reshape(C, 1))
    wvT = np.ascontiguousarray(np.asarray(Wv, np.float32).T.astype(bfnp))
    kv_b = {}
    for b in range(B):
        kvx = kv[:, b].reshape(T, C, NQ).transpose(1, 0, 2).reshape(C, M)
        kvx_bf = kvx.astype(bfnp)
        # kvxT8[p, j, c] = kvx[c, j*128+p]
        kvxT8 = np.ascontiguousarray(
            kvx_bf.astype(np.float32).T.reshape(NCH, 128, C)
            .transpose(1, 0, 2).astype(e4np))
        kv_b[b] = (np.ascontiguousarray(kvx_bf), kvxT8)
    in_maps = []
    for core in range(N_CORES):
        b, t = divmod(core, T)
        qx = np.ascontiguousarray(q[t, b].reshape(C, NQ).astype(bfnp))
        in_maps.append({
            "qx": qx, "aT": aT, "bqk": bqk, "wvT": wvT,
            "kvx": kv_b[b][0], "kvxT8": kv_b[b][1],
        })
    return in_maps


def _assemble(results, bv):
    full = np.empty((B, T * NQ, C), np.float32)
    for core in range(N_CORES):
        b, t = divmod(core, T)
        o2 = results[core]["out2"]
        oT = o2[:, :NQ]                      # [C, NQ] unnormalized
        r = o2[:, NQ:NQ + NRS].T.reshape(NQ) # [p, nb] -> rowsum(nb*128+p)
        full[b, t * NQ:(t + 1) * NQ] = (oT / r[None, :]).T
    full += np.asarray(bv, np.float32)[None, None, :]
    return full


def kernel(query, key_value, Wq, bq, Wk, bk, Wv, bv, **run_kwargs):
    global _NC
    if _NC is None:
        _NC = build_nc()
    in_maps = _prepare_in_maps(query, key_value, Wq, bq, Wk, bk, Wv, bv)
    res = run_bass_kernel_spmd(_NC, in_maps, list(range(N_CORES)), **run_kwargs)
    out = _assemble(res.results, bv)
    if run_kwargs:
        return out, res
    return out


# revision 11
# speedup vs baseline: 1.3381x; 1.3381x over previous
"""Fused cross-attention kernel for Trainium2 (Bass/Tile), 8-core SPMD.

Problem: query/key_value [T=4, B=2, C=128, H=32, W=32] -> tokens [B, N=4096, C],
QKV projections (128x128), full softmax attention over N tokens per batch.

Sharding: core = b*4 + t handles batch b, query tokens [t*1024, (t+1)*1024)
against all 4096 K/V tokens of batch b.

Algebraic restructure (vs. materializing Q/K/V):
  scores:  S^T[m,n] = x_kv[m] . qk[n]   with  qk = (Wk^T Wq) x_q + Wk^T bq,
           prescaled by PRE = scale*log2e*beta/8 (see below)
  output:  O^T = Wv Z / rowsum,  Z[c,n] = sum_m x_kv[m,c] P[m,n]

v2 speedups over the 28us baseline (which was simultaneously at the bf16
PE roofline and the ACT 1-elem/cycle exp roofline):
  1. Z matmul in fp8 e4m3 DoubleRow, m-chunks paired for 256-deep
     contraction: 2 output cols/cycle -> 8192 PE cycles instead of 32768.
     End-to-end rel err with p+kvxT in e4m3 measured 1.15e-2 (gate 2e-2);
     the p-quantization error largely cancels between numerator Z and
     denominator r because r is computed from the same quantized p8.
  2. exp split across TWO engines: ACT (spline exp, 1/cycle @1.2GHz) and a
     custom DVE op EXP8C_ANT: monic cubic in v then 3 squarings
     (8 ALU stages, 1 elem/cycle @0.96GHz).  p = h(v)^8 ~ 2^(8u-5) where
     v = beta*u, u = s*scale*log2e/8; the monic-normalization constant
     beta = (a3*2^(-5/8))^(1/3) is folded into the host-side qk prescale,
     and the 2^-5 fp8 range shift cancels in Z/r.  Poly minimax err 1.9e-3
     (x8 after squarings) is value-deterministic so it also cancels in
     Z/r: measured +0.04% end-to-end.
  3. rowsums as 256 tiny PE matmuls (p8-block stationary, FWL-loaded
     behind the S matmul streams; moving ones8 = 1 col): lands r in PSUM
     [128, 8nb, 32j] as independent single-shot groups (PSUM zero-region
     marking breaks interleaved accumulation groups on HW), reduced to
     [128, 8] by one DVE tensor_reduce at the end.
This rebalances to PE ~18us / ACT ~16us / DVE ~16us.

ldweights-only filler instructions keep the PE busy streak alive for the
HAM clock ramp (p-states 0.65/1.2/2.4 GHz).
"""

import math
from contextlib import ExitStack

import numpy as np
import ml_dtypes

import concourse.bass as bass
import concourse.mybir as mybir
import concourse.tile as tile
from concourse import bacc
from concourse.bass_utils import run_bass_kernel_spmd

F32 = mybir.dt.float32
BF16 = mybir.dt.bfloat16
E4 = mybir.dt.float8e4
AF = mybir.ActivationFunctionType
e4np = ml_dtypes.float8_e4m3
bfnp = ml_dtypes.bfloat16

C = 128        # model dim
NQ = 1024      # query tokens per core
M = 4096       # kv tokens per batch
NCH = M // 128 # m chunks
NPAIR = NCH // 2
T = 4
B = 2
SCALE = 1.0 / math.sqrt(float(C))
N_CORES = 8
NRS = NQ // C  # rowsum cols appended to out2

CFG = dict(
    p_bufs=4,       # p8 pair-tile SBUF buffers
    ps_s_bufs=5,    # score PSUM half-tile buffers ([128,512] = 1 bank each)
    pe_warm=24,     # ldweights warm-ups during the DMA window
    fillers=1,      # ldweights fillers per pair (hold the PE HAM streak)
    head_fill=8,    # ldweights fillers between qk proj and chunk 0
    lookahead=1,    # pairs the Z+rowsum lag behind the S matmuls
    unroll=8,       # kernel bodies per For_i iteration (amortizes barrier)
)

_NC = None

# ---------------- custom DVE exp op ----------------
from concourse.dve_spec import Spec, Src0, C0, C1, C2, lower as _dve_lower
from concourse.dve_uop import DveOpSpec
from concourse.dve_ops import DveOp, OPS as _DVE_OPS, CUSTOM_DVE_SPECS

_FIT_R = 1.13


def _fit_cubic():
    us = np.linspace(-_FIT_R, _FIT_R, 4001)
    tgt = np.exp2(us)
    w = np.ones_like(us)
    for _ in range(60):
        c = np.polyfit(us, tgt, 3, w=w)
        r = np.abs(np.polyval(c, us) / tgt - 1)
        w *= (1 + r / r.max())
        w /= w.mean()
    return [float(x) for x in c]


_A3, _A2, _A1, _A0 = _fit_cubic()
_KAPPA = 2.0 ** (-5.0 / 8.0)
BETA = (_A3 * _KAPPA) ** (1.0 / 3.0)
_EC2 = _KAPPA * _A2 / BETA**2
_EC1 = _KAPPA * _A1 / BETA
_EC0 = _KAPPA * _A0
ACT_SCALE = 8.0 * math.log(2.0) / BETA
ACT_BIAS = -5.0 * math.log(2.0)
PRE = SCALE * math.log2(math.e) * BETA / 8.0  # host fold into qk


def _exp_ref(in0, in1, s0, s1, imm2):
    v = np.asarray(in0, np.float32)
    h = ((v + imm2) * v + s1) * v + s0
    h2 = h * h
    h4 = h2 * h2
    return (h4 * h4).astype(np.float32)


def _make_exp_op():
    import concourse.dve_ops as _do
    name = "EXP8C_ANT"
    for op in _DVE_OPS:
        if op.name == name:
            return op
    v = Src0
    h = ((v + C2) * v + C1) * v + C0
    h2 = h * h
    h4 = h2 * h2
    spec = Spec(body=h4 * h4, reference=_exp_ref)
    shas = {}
    for ver in ("v3", "v4"):
        uops = _dve_lower(spec, ver=ver)
        shas[ver] = DveOpSpec(name=name, opcode=1, uops=uops,
                              rd1_en=False).sha(ver)
    op = DveOp(name, spec, subdim=False, uops_sha=shas)
    _DVE_OPS.append(op)
    CUSTOM_DVE_SPECS[name] = spec
    _do._SUB_OPCODE_FOR_NAME[name] = _do._CUSTOM_DVE_ROW_BASE + len(_DVE_OPS) - 1
    return op


EXP_OP = _make_exp_op()


def build_nc(reps=1, loop_reps=0, **overrides):
    cfg = dict(CFG)
    cfg.update(overrides)

    nc = bacc.Bacc()
    qx = nc.dram_tensor("qx", [C, NQ], BF16, kind="ExternalInput")
    aT = nc.dram_tensor("aT", [C, C], BF16, kind="ExternalInput")
    bqk = nc.dram_tensor("bqk", [C, 1], F32, kind="ExternalInput")
    wvT = nc.dram_tensor("wvT", [C, C], BF16, kind="ExternalInput")
    kvx = nc.dram_tensor("kvx", [C, M], BF16, kind="ExternalInput")
    kvxT8 = nc.dram_tensor("kvxT8", [128, NCH, 128], E4, kind="ExternalInput")
    out2 = nc.dram_tensor("out2", [C, NQ + NRS], F32, kind="ExternalOutput")

    unroll = cfg["unroll"]
    if loop_reps and loop_reps % unroll == 0 and loop_reps >= unroll:
        loop_iters, reps = loop_reps // unroll, reps * unroll
    elif loop_reps:
        loop_iters = loop_reps
    else:
        loop_iters = 0



    with tile.TileContext(nc) as tc, ExitStack() as ctx:
        const = ctx.enter_context(tc.tile_pool(name="const", bufs=1))
        proj = ctx.enter_context(tc.tile_pool(name="proj", bufs=1))
        pwork = ctx.enter_context(tc.tile_pool(name="pwork", bufs=cfg["p_bufs"]))
        owork = ctx.enter_context(tc.tile_pool(name="owork", bufs=2))
        psum = ctx.enter_context(tc.tile_pool(name="psum", bufs=2, space="PSUM"))

        def misc_tile(name):
            # borrow a rotating score-PSUM half-buffer for small/late matmuls
            return psum.tile([128, 512], F32, tag="ps_s",
                             bufs=cfg["ps_s_bufs"], name=name)

        # Constants (gpsimd, no DMA deps). Warm the exp table first so the
        # table load overlaps the input DMAs / NEFF preamble.
        ones_f32 = const.tile([128, 1], F32)
        nc.gpsimd.memset(ones_f32, 1.0)
        warm = const.tile([128, 1], F32)
        nc.scalar.activation(warm, ones_f32, AF.Exp)
        ones8 = const.tile([128, 1], E4)
        nc.gpsimd.memset(ones8, 1.0)
        bias_t = const.tile([128, 1], F32)
        nc.gpsimd.memset(bias_t, ACT_BIAS)
        warm_w = const.tile([128, 128], BF16)
        nc.gpsimd.memset(warm_w, 1.0)

        # PE warm-up: ldweights-only ops to hold the HAM clock ramp.
        for _w in range(cfg["pe_warm"]):
            nc.tensor.ldweights(warm_w)

        # Input DMAs, spread across the SP and ACT HWDGE rings.
        qx_sb = const.tile([C, NQ], BF16)
        nc.sync.dma_start(qx_sb, qx[:])
        aT_sb = const.tile([C, C], BF16)
        nc.sync.dma_start(aT_sb, aT[:])
        bqk_sb = const.tile([C, 1], F32)
        nc.sync.dma_start(bqk_sb, bqk[:])
        wvT_sb = const.tile([C, C], BF16)
        nc.sync.dma_start(wvT_sb, wvT[:])
        kvx_sb = const.tile([C, M], BF16)
        nc.sync.dma_start(kvx_sb[:, 0:1024], kvx[:, 0:1024])
        nc.scalar.dma_start(kvx_sb[:, 1024:2560], kvx[:, 1024:2560])
        nc.scalar.dma_start(kvx_sb[:, 2560:4096], kvx[:, 2560:4096])
        kvxT8_sb = const.tile([128, NCH, 128], E4)
        nc.sync.dma_start(kvxT8_sb[:, 0:16, :], kvxT8[:, 0:16, :])
        nc.scalar.dma_start(kvxT8_sb[:, 16:32, :], kvxT8[:, 16:32, :])

        loop_cm = tc.For_i(0, loop_iters, 1) if loop_iters else None
        if loop_cm is not None:
            loop_cm.__enter__()
        for _rep in range(reps):
            # ---- qk projection: qk = (A*PRE) @ qx + bqk*PRE ----
            psq0 = misc_tile("psq0")
            psq1 = misc_tile("psq1")
            for h, psq in enumerate((psq0, psq1)):
                nc.tensor.matmul(psq, lhsT=aT_sb,
                                 rhs=qx_sb[:, h * 512:(h + 1) * 512],
                                 start=True, stop=True)
            qk_sb = proj.tile([C, NQ], BF16, name="qk_sb")
            with nc.allow_low_precision(reason="bf16 qk tokens"):
                nc.vector.tensor_scalar_add(qk_sb[:, 0:512], psq0, bqk_sb)
                nc.scalar.activation(qk_sb[:, 512:1024], psq1,
                                     AF.Identity, bias=bqk_sb)
            for _f in range(cfg["head_fill"]):
                nc.tensor.ldweights(warm_w)

            # ---- pair loop: 2 m-chunks per iteration; Z+rowsums lag
            # `lookahead` pairs behind the S matmuls so the PE never sits
            # at an instruction waiting for exp. ----
            psz = psum.tile([128, NQ], F32, tag="ps_z", bufs=1, name="psz")
            psr = psum.tile([128, NRS, NCH], F32, tag="ps_r", bufs=1,
                            name="psr")

            def emit_zrs(k, p8):
                # Z: fp8 DoubleRow, 256-deep contraction per pair
                for h in range(2):
                    nc.tensor.matmul(
                        psz[:, h * 512:(h + 1) * 512],
                        lhsT=kvxT8_sb[:, 2 * k:2 * k + 2, :],
                        rhs=p8[:, :, h * 512:(h + 1) * 512],
                        start=(k == 0), stop=(k == NPAIR - 1),
                        perf_mode=mybir.MatmulPerfMode.DoubleRow)
                # rowsums: independent single-shot groups (see docstring)
                for jj in range(2):
                    j = 2 * k + jj
                    for nb in range(NRS):
                        nc.tensor.matmul(
                            psr[:, nb, j:j + 1],
                            lhsT=p8[:, jj, nb * 128:(nb + 1) * 128],
                            rhs=ones8,
                            start=True, stop=True, skip_group_check=True)

            pending = []
            for k in range(NPAIR):
                p8 = pwork.tile([128, 2, NQ], E4, tag="p8",
                                bufs=cfg["p_bufs"])
                for jj in range(2):
                    j = 2 * k + jj
                    # independent per-half PSUM tiles so DVE and ACT free
                    # their score buffers independently
                    pss0 = psum.tile([128, 512], F32, tag="ps_s",
                                     bufs=cfg["ps_s_bufs"])
                    pss1 = psum.tile([128, 512], F32, tag="ps_s",
                                     bufs=cfg["ps_s_bufs"])
                    for h, pss in enumerate((pss0, pss1)):
                        nc.tensor.matmul(
                            pss,
                            lhsT=kvx_sb[:, j * 128:(j + 1) * 128],
                            rhs=qk_sb[:, h * 512:(h + 1) * 512],
                            start=True, stop=True)
                    with nc.allow_low_precision(reason="fp8 p"):
                        nc.vector._custom_dve(
                            EXP_OP, out=p8[:, jj, 0:512], in0=pss0,
                            s0=_EC0, s1=_EC1, imm2=_EC2)
                        nc.scalar.activation(p8[:, jj, 512:NQ],
                                             pss1, AF.Exp,
                                             scale=ACT_SCALE, bias=bias_t)
                pending.append((k, p8))
                if len(pending) > cfg["lookahead"]:
                    emit_zrs(*pending.pop(0))
                if cfg["fillers"]:
                    for _f in range(cfg["fillers"]):
                        # reading p8 pins the filler to this pair
                        nc.tensor.ldweights(p8[:, 0, 0:128])
            for item in pending:
                emit_zrs(*item)

            # ---- O^T = Wv @ Z (unnormalized; host divides by rowsums),
            # quarter-split so drain/matmul/copy/DMA pipeline across engines.
            # Rowsums reduced into cols [NQ, NQ+8) of the same output. ----
            z_sb = proj.tile([C, NQ], BF16, name="z_sb")
            pso0 = misc_tile("pso0")
            pso1 = misc_tile("pso1")
            o_sb = owork.tile([128, NQ + NRS], F32, tag="o_sb", bufs=2,
                              name="o_sb")
            nc.vector.tensor_reduce(o_sb[:, NQ:NQ + NRS], psr[:, :, :],
                                    mybir.AxisListType.X, mybir.AluOpType.add)
            for q in range(4):
                qs = slice(q * 256, (q + 1) * 256)
                pso = (pso0, pso1)[q // 2]
                ps = slice((q % 2) * 256, (q % 2) * 256 + 256)
                with nc.allow_low_precision(reason="bf16 z"):
                    nc.vector.tensor_copy(z_sb[:, qs], psz[:, qs])
                nc.tensor.matmul(pso[:, ps], lhsT=wvT_sb, rhs=z_sb[:, qs],
                                 start=True, stop=True)
                nc.scalar.copy(o_sb[:, qs], pso[:, ps])
                if q < 3:
                    nc.sync.dma_start(out2[:, qs], o_sb[:, qs])
                else:
                    nc.sync.dma_start(out2[:, 768:NQ + NRS],
                                      o_sb[:, 768:NQ + NRS])
        if loop_cm is not None:
            loop_cm.__exit__(None, None, None)
    nc.compile()
    return nc


def _prepare_in_maps(query, key_value, Wq, bq, Wk, bk, Wv, bv):
    q = np.asarray(query, np.float32)
    kv = np.asarray(key_value, np.float32)
    Wq64 = np.asarray(Wq, np.float64)
    Wk64 = np.asarray(Wk, np.float64)
    aT = np.ascontiguousarray(((Wq64.T @ Wk64) * PRE).astype(bfnp))
    bqk = np.ascontiguousarray(
        ((Wk64.T @ np.asarray(bq, np.float64)) * PRE
         ).astype(np.float32).reshape(C, 1))
    wvT = np.ascontiguousarray(np.asarray(Wv, np.float32).T.astype(bfnp))
    kv_b = {}
    for b in range(B):
        kvx = kv[:, b].reshape(T, C, NQ).transpose(1, 0, 2).reshape(C, M)
        kvx_bf = kvx.astype(bfnp)
        # kvxT8[p, j, c] = kvx[c, j*128+p]
        kvxT8 = np.ascontiguousarray(
            kvx_bf.astype(np.float32).T.reshape(NCH, 128, C)
            .transpose(1, 0, 2).astype(e4np))
        kv_b[b] = (np.ascontiguousarray(kvx_bf), kvxT8)
    in_maps = []
    for core in range(N_CORES):
        b, t = divmod(core, T)
        qx = np.ascontiguousarray(q[t, b].reshape(C, NQ).astype(bfnp))
        in_maps.append({
            "qx": qx, "aT": aT, "bqk": bqk, "wvT": wvT,
            "kvx": kv_b[b][0], "kvxT8": kv_b[b][1],
        })
    return in_maps


def _assemble(results, bv):
    full = np.empty((B, T * NQ, C), np.float32)
    for core in range(N_CORES):
        b, t = divmod(core, T)
        o2 = results[core]["out2"]
        oT = o2[:, :NQ]                      # [C, NQ] unnormalized
        r = o2[:, NQ:NQ + NRS].T.reshape(NQ) # [p, nb] -> rowsum(nb*128+p)
        full[b, t * NQ:(t + 1) * NQ] = (oT / r[None, :]).T
    full += np.asarray(bv, np.float32)[None, None, :]
    return full


def kernel(query, key_value, Wq, bq, Wk, bk, Wv, bv, **run_kwargs):
    global _NC
    if _NC is None:
        _NC = build_nc()
    in_maps = _prepare_in_maps(query, key_value, Wq, bq, Wk, bk, Wv, bv)
    res = run_bass_kernel_spmd(_NC, in_maps, list(range(N_CORES)), **run_kwargs)
    out = _assemble(res.results, bv)
    if run_kwargs:
        return out, res
    return out


# revision 18
# speedup vs baseline: 1.3833x; 1.0338x over previous
"""Fused cross-attention kernel for Trainium2 (Bass/Tile), 8-core SPMD.

Problem: query/key_value [T=4, B=2, C=128, H=32, W=32] -> tokens [B, N=4096, C],
QKV projections (128x128), full softmax attention over N tokens per batch.

Sharding: core = b*4 + t handles batch b, query tokens [t*1024, (t+1)*1024)
against all 4096 K/V tokens of batch b.

Algebraic restructure (vs. materializing Q/K/V):
  scores:  S^T[m,n] = x_kv[m] . qk[n]   with  qk = (Wk^T Wq) x_q + Wk^T bq,
           prescaled by PRE = scale*log2e*beta/8 (see below)
  output:  O^T = Wv Z / rowsum,  Z[c,n] = sum_m x_kv[m,c] P[m,n]

v2 speedups over the 28us baseline (which was simultaneously at the bf16
PE roofline and the ACT 1-elem/cycle exp roofline):
  1. Z matmul in fp8 e4m3 DoubleRow, m-chunks paired for 256-deep
     contraction: 2 output cols/cycle -> 8192 PE cycles instead of 32768.
     End-to-end rel err with p+kvxT in e4m3 measured 1.15e-2 (gate 2e-2);
     the p-quantization error largely cancels between numerator Z and
     denominator r because r is computed from the same quantized p8.
  2. exp split across TWO engines: ACT (spline exp, 1/cycle @1.2GHz) and a
     custom DVE op EXP8C_ANT: monic cubic in v then 3 squarings
     (8 ALU stages, 1 elem/cycle @0.96GHz).  p = h(v)^8 ~ 2^(8u-5) where
     v = beta*u, u = s*scale*log2e/8; the monic-normalization constant
     beta = (a3*2^(-5/8))^(1/3) is folded into the host-side qk prescale,
     and the 2^-5 fp8 range shift cancels in Z/r.  Poly minimax err 1.9e-3
     (x8 after squarings) is value-deterministic so it also cancels in
     Z/r: measured +0.04% end-to-end.
  3. rowsums as 256 tiny PE matmuls (p8-block stationary, FWL-loaded
     behind the S matmul streams; moving ones8 = 1 col): lands r in PSUM
     [128, 8nb, 32j] as independent single-shot groups (PSUM zero-region
     marking breaks interleaved accumulation groups on HW), reduced to
     [128, 8] by one DVE tensor_reduce at the end.

Measured per-instruction engine costs (HW loop-slope microbench):
ACT exp [128,512] psum->fp8 643ns, [128,1024] 1047ns; DVE custom
682/1191ns; so the engine-busy floor is ~22.7us (32+32 512-col exp
instrs split ACT/DVE + drains) vs the baseline's ~27.3us ACT-only exp
floor.  Remaining gap to that floor is fused-matmul weight loads
serializing behind exp sem-waits (Z DoubleRow loads 256 cols non-FWL,
rowsum FWL loads) plus For_i loop-boundary effects.

ldweights-only filler instructions keep the PE busy streak alive for the
HAM clock ramp (p-states 0.65/1.2/2.4 GHz).
"""

import math
from contextlib import ExitStack

import numpy as np
import ml_dtypes

import concourse.bass as bass
import concourse.mybir as mybir
import concourse.tile as tile
from concourse import bacc
from concourse.bass_utils import run_bass_kernel_spmd

F32 = mybir.dt.float32
BF16 = mybir.dt.bfloat16
E4 = mybir.dt.float8e4
AF = mybir.ActivationFunctionType
e4np = ml_dtypes.float8_e4m3
bfnp = ml_dtypes.bfloat16

C = 128        # model dim
NQ = 1024      # query tokens per core
M = 4096       # kv tokens per batch
NCH = M // 128 # m chunks
NPAIR = NCH // 2
T = 4
B = 2
SCALE = 1.0 / math.sqrt(float(C))
N_CORES = 8
NRS = NQ // C  # rowsum cols appended to out2

CFG = dict(
    p_bufs=4,       # p8 pair-tile SBUF buffers
    ps_s_bufs=5,    # score PSUM half-tile buffers ([128,512] = 1 bank each)
    pe_warm=24,     # ldweights warm-ups during the DMA window
    fillers=1,      # ldweights fillers per pair (hold the PE HAM streak)
    head_fill=8,    # ldweights fillers between qk proj and chunk 0
    lookahead=1,    # pairs the Z+rowsum lag behind the S matmuls
    unroll=8,       # kernel bodies per For_i iteration (amortizes barrier)
    no_rowsum=False,   # A/B: skip rowsum matmuls (output r invalid)
    act_exp_only=False,  # A/B: all exp on ACT (two 512-col instrs/chunk)
)

_NC = None

# ---------------- custom DVE exp op ----------------
from concourse.dve_spec import Spec, Src0, C0, C1, C2, lower as _dve_lower
from concourse.dve_uop import DveOpSpec
from concourse.dve_ops import DveOp, OPS as _DVE_OPS, CUSTOM_DVE_SPECS

_FIT_R = 1.13


def _fit_cubic():
    us = np.linspace(-_FIT_R, _FIT_R, 4001)
    tgt = np.exp2(us)
    w = np.ones_like(us)
    for _ in range(60):
        c = np.polyfit(us, tgt, 3, w=w)
        r = np.abs(np.polyval(c, us) / tgt - 1)
        w *= (1 + r / r.max())
        w /= w.mean()
    return [float(x) for x in c]


_A3, _A2, _A1, _A0 = _fit_cubic()
_KAPPA = 2.0 ** (-5.0 / 8.0)
BETA = (_A3 * _KAPPA) ** (1.0 / 3.0)
_EC2 = _KAPPA * _A2 / BETA**2
_EC1 = _KAPPA * _A1 / BETA
_EC0 = _KAPPA * _A0
ACT_SCALE = 8.0 * math.log(2.0) / BETA
ACT_BIAS = -5.0 * math.log(2.0)
PRE = SCALE * math.log2(math.e) * BETA / 8.0  # host fold into qk


def _exp_ref(in0, in1, s0, s1, imm2):
    v = np.asarray(in0, np.float32)
    h = ((v + imm2) * v + s1) * v + s0
    h2 = h * h
    h4 = h2 * h2
    return (h4 * h4).astype(np.float32)


def _make_exp_op():
    import concourse.dve_ops as _do
    name = "EXP8C_ANT"
    for op in _DVE_OPS:
        if op.name == name:
            return op
    v = Src0
    h = ((v + C2) * v + C1) * v + C0
    h2 = h * h
    h4 = h2 * h2
    spec = Spec(body=h4 * h4, reference=_exp_ref)
    shas = {}
    for ver in ("v3", "v4"):
        uops = _dve_lower(spec, ver=ver)
        shas[ver] = DveOpSpec(name=name, opcode=1, uops=uops,
                              rd1_en=False).sha(ver)
    op = DveOp(name, spec, subdim=False, uops_sha=shas)
    _DVE_OPS.append(op)
    CUSTOM_DVE_SPECS[name] = spec
    _do._SUB_OPCODE_FOR_NAME[name] = _do._CUSTOM_DVE_ROW_BASE + len(_DVE_OPS) - 1
    return op


EXP_OP = _make_exp_op()


def build_nc(reps=1, loop_reps=0, **overrides):
    cfg = dict(CFG)
    cfg.update(overrides)

    nc = bacc.Bacc()
    qx = nc.dram_tensor("qx", [C, NQ], BF16, kind="ExternalInput")
    aT = nc.dram_tensor("aT", [C, C], BF16, kind="ExternalInput")
    bqk = nc.dram_tensor("bqk", [C, 1], F32, kind="ExternalInput")
    wvT = nc.dram_tensor("wvT", [C, C], BF16, kind="ExternalInput")
    kvx = nc.dram_tensor("kvx", [C, M], BF16, kind="ExternalInput")
    kvxT8 = nc.dram_tensor("kvxT8", [128, NCH, 128], E4, kind="ExternalInput")
    out2 = nc.dram_tensor("out2", [C, NQ + NRS], F32, kind="ExternalOutput")

    unroll = cfg["unroll"]
    if loop_reps and loop_reps % unroll == 0 and loop_reps >= unroll:
        loop_iters, reps = loop_reps // unroll, reps * unroll
    elif loop_reps:
        loop_iters = loop_reps
    else:
        loop_iters = 0



    with tile.TileContext(nc) as tc, ExitStack() as ctx:
        const = ctx.enter_context(tc.tile_pool(name="const", bufs=1))
        proj = ctx.enter_context(tc.tile_pool(name="proj", bufs=1))
        pwork = ctx.enter_context(tc.tile_pool(name="pwork", bufs=cfg["p_bufs"]))
        owork = ctx.enter_context(tc.tile_pool(name="owork", bufs=2))
        psum = ctx.enter_context(tc.tile_pool(name="psum", bufs=2, space="PSUM"))

        def misc_tile(name):
            # borrow a rotating score-PSUM half-buffer for small/late matmuls
            return psum.tile([128, 512], F32, tag="ps_s",
                             bufs=cfg["ps_s_bufs"], name=name)

        # Constants (gpsimd, no DMA deps). Warm the exp table first so the
        # table load overlaps the input DMAs / NEFF preamble.
        ones_f32 = const.tile([128, 1], F32)
        nc.gpsimd.memset(ones_f32, 1.0)
        warm = const.tile([128, 1], F32)
        nc.scalar.activation(warm, ones_f32, AF.Exp)
        ones8 = const.tile([128, 1], E4)
        nc.gpsimd.memset(ones8, 1.0)
        bias_t = const.tile([128, 1], F32)
        nc.gpsimd.memset(bias_t, ACT_BIAS)
        warm_w = const.tile([128, 128], BF16)
        nc.gpsimd.memset(warm_w, 1.0)

        # PE warm-up: ldweights-only ops to hold the HAM clock ramp.
        for _w in range(cfg["pe_warm"]):
            nc.tensor.ldweights(warm_w)

        # Input DMAs, spread across the SP and ACT HWDGE rings.
        qx_sb = const.tile([C, NQ], BF16)
        nc.sync.dma_start(qx_sb, qx[:])
        aT_sb = const.tile([C, C], BF16)
        nc.sync.dma_start(aT_sb, aT[:])
        bqk_sb = const.tile([C, 1], F32)
        nc.sync.dma_start(bqk_sb, bqk[:])
        wvT_sb = const.tile([C, C], BF16)
        nc.sync.dma_start(wvT_sb, wvT[:])
        kvx_sb = const.tile([C, M], BF16)
        nc.sync.dma_start(kvx_sb[:, 0:1024], kvx[:, 0:1024])
        nc.scalar.dma_start(kvx_sb[:, 1024:2560], kvx[:, 1024:2560])
        nc.scalar.dma_start(kvx_sb[:, 2560:4096], kvx[:, 2560:4096])
        kvxT8_sb = const.tile([128, NCH, 128], E4)
        nc.sync.dma_start(kvxT8_sb[:, 0:16, :], kvxT8[:, 0:16, :])
        nc.scalar.dma_start(kvxT8_sb[:, 16:32, :], kvxT8[:, 16:32, :])

        loop_cm = tc.For_i(0, loop_iters, 1) if loop_iters else None
        if loop_cm is not None:
            loop_cm.__enter__()
        for _rep in range(reps):
            # ---- qk projection: qk = (A*PRE) @ qx + bqk*PRE ----
            psq0 = misc_tile("psq0")
            psq1 = misc_tile("psq1")
            for h, psq in enumerate((psq0, psq1)):
                nc.tensor.matmul(psq, lhsT=aT_sb,
                                 rhs=qx_sb[:, h * 512:(h + 1) * 512],
                                 start=True, stop=True)
            qk_sb = proj.tile([C, NQ], BF16, name="qk_sb")
            with nc.allow_low_precision(reason="bf16 qk tokens"):
                nc.vector.tensor_scalar_add(qk_sb[:, 0:512], psq0, bqk_sb)
                nc.scalar.activation(qk_sb[:, 512:1024], psq1,
                                     AF.Identity, bias=bqk_sb)
            for _f in range(cfg["head_fill"]):
                nc.tensor.ldweights(warm_w)

            # ---- pair loop: 2 m-chunks per iteration; Z+rowsums lag
            # `lookahead` pairs behind the S matmuls so the PE never sits
            # at an instruction waiting for exp. ----
            psz = psum.tile([128, NQ], F32, tag="ps_z", bufs=1, name="psz")
            psr = psum.tile([128, NRS, NCH], F32, tag="ps_r", bufs=1,
                            name="psr")

            def rs_batch(k, p8, jj, nbs):
                # rowsums: independent single-shot groups (see docstring);
                # interleaved between S streams so the 128-col FWL weight
                # loads prefetch during long matmul streams.
                if cfg["no_rowsum"]:
                    if k == 0 and jj == 0:
                        for nb in nbs:
                            nc.tensor.matmul(
                                psr[:, nb, 0:1],
                                lhsT=p8[:, 0, nb * 128:(nb + 1) * 128],
                                rhs=ones8, start=True, stop=True,
                                skip_group_check=True)
                    return
                j = 2 * k + jj
                for nb in nbs:
                    nc.tensor.matmul(
                        psr[:, nb, j:j + 1],
                        lhsT=p8[:, jj, nb * 128:(nb + 1) * 128],
                        rhs=ones8,
                        start=True, stop=True, skip_group_check=True)

            def emit_z(k, p8, h):
                # Z: fp8 DoubleRow, 256-deep contraction per pair
                nc.tensor.matmul(
                    psz[:, h * 512:(h + 1) * 512],
                    lhsT=kvxT8_sb[:, 2 * k:2 * k + 2, :],
                    rhs=p8[:, :, h * 512:(h + 1) * 512],
                    start=(k == 0), stop=(k == NPAIR - 1),
                    perf_mode=mybir.MatmulPerfMode.DoubleRow)

            pending = []
            for k in range(NPAIR):
                p8 = pwork.tile([128, 2, NQ], E4, tag="p8",
                                bufs=cfg["p_bufs"])
                prev = pending.pop(0) if len(pending) >= cfg["lookahead"] \
                    else None
                for jj in range(2):
                    j = 2 * k + jj
                    # independent per-half PSUM tiles so DVE and ACT free
                    # their score buffers independently
                    pss0 = psum.tile([128, 512], F32, tag="ps_s",
                                     bufs=cfg["ps_s_bufs"])
                    pss1 = psum.tile([128, 512], F32, tag="ps_s",
                                     bufs=cfg["ps_s_bufs"])
                    nc.tensor.matmul(pss0,
                                     lhsT=kvx_sb[:, j * 128:(j + 1) * 128],
                                     rhs=qk_sb[:, 0:512],
                                     start=True, stop=True)
                    if prev is not None:
                        rs_batch(prev[0], prev[1], jj, range(0, 4))
                    nc.tensor.matmul(pss1,
                                     lhsT=kvx_sb[:, j * 128:(j + 1) * 128],
                                     rhs=qk_sb[:, 512:1024],
                                     start=True, stop=True)
                    if prev is not None:
                        rs_batch(prev[0], prev[1], jj, range(4, 8))
                    with nc.allow_low_precision(reason="fp8 p"):
                        if cfg["act_exp_only"]:
                            nc.scalar.activation(p8[:, jj, 0:512],
                                                 pss0, AF.Exp,
                                                 scale=ACT_SCALE, bias=bias_t)
                        else:
                            nc.vector._custom_dve(
                                EXP_OP, out=p8[:, jj, 0:512], in0=pss0,
                                s0=_EC0, s1=_EC1, imm2=_EC2)
                        nc.scalar.activation(p8[:, jj, 512:NQ],
                                             pss1, AF.Exp,
                                             scale=ACT_SCALE, bias=bias_t)
                    if prev is not None and jj == 1:
                        emit_z(prev[0], prev[1], 0)
                        emit_z(prev[0], prev[1], 1)
                pending.append((k, p8))
                if cfg["fillers"]:
                    for _f in range(cfg["fillers"]):
                        # reading p8 pins the filler to this pair
                        nc.tensor.ldweights(p8[:, 0, 0:128])
            for (kk, pp) in pending:
                for jj in range(2):
                    rs_batch(kk, pp, jj, range(0, NRS))
                emit_z(kk, pp, 0)
                emit_z(kk, pp, 1)

            # ---- O^T = Wv @ Z (unnormalized; host divides by rowsums),
            # quarter-split so drain/matmul/copy/DMA pipeline across engines.
            # Rowsums reduced into cols [NQ, NQ+8) of the same output. ----
            z_sb = proj.tile([C, NQ], BF16, name="z_sb")
            pso0 = misc_tile("pso0")
            pso1 = misc_tile("pso1")
            o_sb = owork.tile([128, NQ + NRS], F32, tag="o_sb", bufs=2,
                              name="o_sb")
            nc.vector.tensor_reduce(o_sb[:, NQ:NQ + NRS], psr[:, :, :],
                                    mybir.AxisListType.X, mybir.AluOpType.add)
            for q in range(4):
                qs = slice(q * 256, (q + 1) * 256)
                pso = (pso0, pso1)[q // 2]
                ps = slice((q % 2) * 256, (q % 2) * 256 + 256)
                with nc.allow_low_precision(reason="bf16 z"):
                    nc.vector.tensor_copy(z_sb[:, qs], psz[:, qs])
                nc.tensor.matmul(pso[:, ps], lhsT=wvT_sb, rhs=z_sb[:, qs],
                                 start=True, stop=True)
                nc.scalar.copy(o_sb[:, qs], pso[:, ps])
                if q < 3:
                    nc.sync.dma_start(out2[:, qs], o_sb[:, qs])
                else:
                    nc.sync.dma_start(out2[:, 768:NQ + NRS],
                                      o_sb[:, 768:NQ + NRS])
        if loop_cm is not None:
            loop_cm.__exit__(None, None, None)
    nc.compile()
    return nc


def _prepare_in_maps(query, key_value, Wq, bq, Wk, bk, Wv, bv):
    q = np.asarray(query, np.float32)
    kv = np.asarray(key_value, np.float32)
    Wq64 = np.asarray(Wq, np.float64)
    Wk64 = np.asarray(Wk, np.float64)
    aT = np.ascontiguousarray(((Wq64.T @ Wk64) * PRE).astype(bfnp))
    bqk = np.ascontiguousarray(
        ((Wk64.T @ np.asarray(bq, np.float64)) * PRE
         ).astype(np.float32).reshape(C, 1))
    wvT = np.ascontiguousarray(np.asarray(Wv, np.float32).T.astype(bfnp))
    kv_b = {}
    for b in range(B):
        kvx = kv[:, b].reshape(T, C, NQ).transpose(1, 0, 2).reshape(C, M)
        kvx_bf = kvx.astype(bfnp)
        # kvxT8[p, j, c] = kvx[c, j*128+p]
        kvxT8 = np.ascontiguousarray(
            kvx_bf.astype(np.float32).T.reshape(NCH, 128, C)
            .transpose(1, 0, 2).astype(e4np))
        kv_b[b] = (np.ascontiguousarray(kvx_bf), kvxT8)
    in_maps = []
    for core in range(N_CORES):
        b, t = divmod(core, T)
        qx = np.ascontiguousarray(q[t, b].reshape(C, NQ).astype(bfnp))
        in_maps.append({
            "qx": qx, "aT": aT, "bqk": bqk, "wvT": wvT,
            "kvx": kv_b[b][0], "kvxT8": kv_b[b][1],
        })
    return in_maps


def _assemble(results, bv):
    full = np.empty((B, T * NQ, C), np.float32)
    for core in range(N_CORES):
        b, t = divmod(core, T)
        o2 = results[core]["out2"]
        oT = o2[:, :NQ]                      # [C, NQ] unnormalized
        r = o2[:, NQ:NQ + NRS].T.reshape(NQ) # [p, nb] -> rowsum(nb*128+p)
        full[b, t * NQ:(t + 1) * NQ] = (oT / r[None, :]).T
    full += np.asarray(bv, np.float32)[None, None, :]
    return full


def kernel(query, key_value, Wq, bq, Wk, bk, Wv, bv, **run_kwargs):
    global _NC
    if _NC is None:
        _NC = build_nc()
    in_maps = _prepare_in_maps(query, key_value, Wq, bq, Wk, bk, Wv, bv)
    res = run_bass_kernel_spmd(_NC, in_maps, list(range(N_CORES)), **run_kwargs)
    out = _assemble(res.results, bv)
    if run_kwargs:
        return out, res
    return out
